# revision 16
# baseline (speedup 1.0000x reference)
"""ChebConv (K=6) message-passing kernel for 8 Trainium2 NeuronCores.

Math: the reference's GraphNetwork pass multiplies each node's features by a
per-node scalar s = (deg - in_w) / max(deg) (deg = segment_sum(edges, senders),
in_w = segment_sum(edges, receivers)), and the Chebyshev recurrence
Tx_k = 2*Tx_{k-1} - Tx_{k-2} stays rank-1 per node: Tx_k = (1 + k*(s-1)) * x.
Hence
    out = X @ WA + s * (X @ WB) + b_tot
with WA = sum_k (1-k) Wk[k], WB = sum_k k Wk[k], b_tot = sum_k bk[k] + bias.

Sharding: nodes are block-sharded over 8 cores (12500 each, padded to 12544).
Edges are routed on the host (index permutation + zero fill only, no float
arithmetic) to the core owning their sender (for deg) / receiver (for in_w).

Two launches (an in-kernel AllReduce costs ~60us of comm-subsystem init, far
more than a second NEFF):
  A: edge kernel -- segment sums as PE mask-matmuls.  Edges are packed on the
     host into fp8 columns of 128 slots; nodes are degree-classed (slot count
     rounded to a multiple of 8) so a column holds k = 128//class nodes at
     fixed offsets.  One [128,32] 0/1 mask per class turns a 512-column chunk
     into per-node sums via a single matmul per PSUM quadrant
     (tile_position col-tiling).  DVE max-reduce + gpsimd partition reduce
     give the per-core max(deg).
  host: m = max of the 8 device partial maxima (selection only); deg/in_w
     values are host-permuted (selection) from packed order to node order.
  B: main kernel -- s = (deg-inw)*recip(m); transposed layout outT[fo, n]:
     psF = WA^T @ X + WB^T @ (s*X) accumulated in PSUM, evacuated by ACT with
     per-partition bias; s broadcast along partitions via an fp8 DRAM
     broadcast read (srep).
"""

import sys

sys.path.insert(0, "/opt/trn_rl_repo")

import numpy as np
import ml_dtypes

import concourse.bacc as bacc
import concourse.bass as bass
import concourse.mybir as mybir
import concourse.tile as tile
from concourse import bass_isa, masks
from concourse.bass_utils import run_bass_kernel_spmd

N_NODES = 100000
F = 128
KCH = 6
NCORES = 8
NPC = N_NODES // NCORES       # 12500 nodes per core
T = (NPC + 127) // 128        # 98 node tiles per core
NPAD = T * 128                # 12544 (cols 12500.. are zero padding)

f32 = mybir.dt.float32
bf16 = mybir.dt.bfloat16
fp16 = mybir.dt.float16
fp8 = mybir.dt.float8e4

np_fp8 = ml_dtypes.float8_e4m3fn

# test.py knobs (harness never touches these)
TRACE = False
LAST = {}

PW = 512                      # PSUM chunk width (one bank of f32)

_prog_cache = {}


# --------------------------------------------------------------------------
# host-side edge packing for the PE segment-sum (permutation + zero-fill only)
# --------------------------------------------------------------------------

class _CommonPlan:
    """Shared (all-cores) packing layout for one buffer (sender- or
    receiver-keyed edges).

    Column layout: a column has 128 slots; it holds k = 128//cls nodes of the
    same degree-class cls (cls = ceil8(cnt)), node i at slot rows
    [i*cls, i*cls + cnt).  Per-class column counts are the max over cores so
    one program serves all cores (unused columns stay zero).  Columns are
    grouped into pieces of <= PW columns (one matmul each), pieces into
    chunks of <= 4 (PSUM quadrants).
    """

    def __init__(self, cls_per_core):
        classes = set()
        for cls in cls_per_core:
            classes.update(int(c) for c in np.unique(cls))
        self.classes = sorted(classes)
        self.colbase_cl = []       # virtual (class-contiguous) column base
        vcol0 = 0
        pieces = []  # [cls_idx, vcol0, ncols]
        for ci, cl in enumerate(self.classes):
            k = 128 // cl
            n = max(int((cls == cl).sum()) for cls in cls_per_core)
            ncol = (n + k - 1) // k
            self.colbase_cl.append(vcol0)
            p0 = 0
            while p0 < ncol:
                w = min(PW, ncol - p0)
                pieces.append([ci, vcol0 + p0, w])
                p0 += w
            vcol0 += ncol
        self.nvcols = vcol0
        # chunks: group pieces (desc by width) into groups of 4; every piece
        # occupies a full chunk-width data stripe (zero-padded) so the matmul
        # covers the whole PSUM quadrant (stale-PSUM-free max/evac).
        pieces.sort(key=lambda p: -p[2])
        self.chunks = []   # list of (W, [(cls_idx, padded_base, vcol0, w), ...])
        ncols = 0
        for i in range(0, len(pieces), 4):
            grp = pieces[i : i + 4]
            W = grp[0][2]
            g2 = []
            for ci, v0, w in grp:
                g2.append((ci, ncols, v0, w))
                ncols += W
            self.chunks.append((W, g2))
        self.ncols = ncols
        self.evac_cols = sum(W for W, _ in self.chunks)
        # piece lookup tables for vcol -> padded data col
        self._pv0 = np.array([p[2] for _, g in self.chunks for p in g], np.int64)
        self._ppb = np.array([p[1] for _, g in self.chunks for p in g], np.int64)

    def _v2d(self, vcol):
        """Map virtual cols to padded data cols (vectorized)."""
        # piece index: each piece covers [v0, v0+w) and pieces are disjoint
        order = np.argsort(self._pv0)
        v0s = self._pv0[order]
        pbs = self._ppb[order]
        pi = np.searchsorted(v0s, vcol, side="right") - 1
        return pbs[pi] + (vcol - v0s[pi])

    def fill(self, data, colbase, ln, vals, NPCL):
        """Scatter edge values into data[:, colbase:...]; returns per-node
        (evac_partition, evac_col) for reading packed sums back."""
        cnt = np.bincount(ln, minlength=NPCL)
        assert cnt.max() <= 128, f"node degree {cnt.max()} > 128 unsupported"
        cls = np.maximum(((np.maximum(cnt, 1) + 7) // 8) * 8, 8)
        node_vcol = np.zeros(NPCL, np.int64)
        node_row = np.zeros(NPCL, np.int64)
        for ci, cl in enumerate(self.classes):
            nodes = np.nonzero(cls == cl)[0]
            if not len(nodes):
                continue
            k = 128 // cl
            j = np.arange(len(nodes))
            node_vcol[nodes] = self.colbase_cl[ci] + j // k
            node_row[nodes] = (j % k) * cl
        node_dcol = self._v2d(node_vcol)
        order = np.argsort(ln, kind="stable")
        se = ln[order]
        sv = vals[order]
        first = np.concatenate(([0], np.cumsum(cnt)[:-1]))
        slot = np.arange(len(se), dtype=np.int64) - first[se]
        data[node_row[se] + slot, colbase + node_dcol[se]] = sv.astype(data.dtype)
        # evac mapping: chunk occupies evac cols [off, off+W); node value at
        # partition 32*quad + slot_idx, col off + (vcol - piece_vcol0)
        part = np.zeros(NPCL, np.int64)
        col = np.zeros(NPCL, np.int64)
        off = 0
        for W, grp in self.chunks:
            for quad, (ci, pb, v0, w) in enumerate(grp):
                cl = self.classes[ci]
                sel = np.nonzero((cls == cl) & (node_vcol >= v0) & (node_vcol < v0 + w))[0]
                part[sel] = 32 * quad + node_row[sel] // cl
                col[sel] = off + node_vcol[sel] - v0
            off += W
        return part, col


def _build_edge_program(planS, planR):
    """Launch A: deg/in_w per-node sums via mask matmuls + max(deg)."""
    nc = bacc.Bacc("TRN2", target_bir_lowering=False, debug=False,
                   num_devices=NCORES)
    A = mybir.AluOpType
    X = mybir.AxisListType.X

    ncls = len(planS.classes) + len(planR.classes)
    MASKW = 32 * ncls
    NCOLS = planS.ncols + planR.ncols
    EVC = planS.evac_cols + planR.evac_cols
    nsend = len(planS.chunks)

    ed_d = nc.dram_tensor("ed", [128, MASKW + NCOLS], fp8, kind="ExternalInput")
    degw_d = nc.dram_tensor("degw", [128, EVC], fp16, kind="ExternalOutput")
    pmax_d = nc.dram_tensor("pmax", [1, 1], f32, kind="ExternalOutput")

    # DMA split points for the edge data (4 pieces, chunk-aligned)
    allchunks = [(W, grp, True) for W, grp in planS.chunks] + \
                [(W, grp, False) for W, grp in planR.chunks]

    with tile.TileContext(nc) as tc:
        with (
            tc.tile_pool(name="ed", bufs=1) as edp,
            tc.tile_pool(name="small", bufs=1) as smallp,
            tc.tile_pool(name="ps", bufs=min(len(allchunks), 6), space="PSUM") as psp,
        ):
            ed_sb = edp.tile([128, MASKW + NCOLS], fp8)
            nc.sync.dma_start(ed_sb[:, :MASKW], ed_d[:, :MASKW])
            # split the value region into ~4 DMAs at chunk data boundaries
            bounds = [MASKW]
            # data columns are laid out class-major; chunk pieces reference
            # absolute cols.  Just split evenly into 4 and let tile dep-track.
            step = (NCOLS + 3) // 4
            for b0 in range(0, NCOLS, step):
                b1 = min(NCOLS, b0 + step)
                nc.sync.dma_start(ed_sb[:, MASKW + b0 : MASKW + b1],
                                  ed_d[:, MASKW + b0 : MASKW + b1])

            degsb = smallp.tile([128, EVC], fp16)
            dmax = smallp.tile([128, nsend], f32)
            nc.vector.memset(dmax[:, :], 0.0)

            off = 0
            sidx = 0
            send_end = 0
            for kc, (W, grp, is_send) in enumerate(allchunks):
                ps = psp.tile([128, PW], f32, tag="ps")
                nq = len(grp)
                for quad, (ci, pb, v0, w) in enumerate(grp):
                    cbase = ci if is_send else len(planS.classes) + ci
                    dcol = (0 if is_send else planS.ncols) + pb
                    nc.tensor.matmul(
                        ps[32 * quad : 32 * quad + 32, 0:W],
                        ed_sb[:, cbase * 32 : cbase * 32 + 32],
                        ed_sb[:, MASKW + dcol : MASKW + dcol + W],
                        start=True, stop=True,
                        tile_position=(0, 32 * quad),
                    )
                if is_send:
                    nc.vector.tensor_reduce(dmax[0 : 32 * nq, sidx : sidx + 1],
                                            ps[0 : 32 * nq, 0:W], axis=X, op=A.max)
                    sidx += 1
                # alternate evacuation engine so neither DVE nor ACT binds
                if kc % 2 == 0:
                    nc.scalar.activation(degsb[0 : 32 * nq, off : off + W],
                                         ps[0 : 32 * nq, 0:W],
                                         mybir.ActivationFunctionType.Identity,
                                         bias=0.0, scale=1.0)
                else:
                    nc.vector.tensor_copy(degsb[0 : 32 * nq, off : off + W],
                                          ps[0 : 32 * nq, 0:W])
                off += W
                if is_send and sidx == nsend:
                    # sender side done: resolve the max + ship the sender half
                    # early so only the receiver tail trails the last chunk
                    send_end = off
                    gmax = smallp.tile([128, 1], f32)
                    nc.vector.tensor_reduce(gmax[:, :], dmax[:, :], axis=X, op=A.max)
                    pmax = smallp.tile([128, 1], f32)
                    nc.gpsimd.partition_all_reduce(pmax[:, :], gmax[:, :], channels=128,
                                                   reduce_op=bass_isa.ReduceOp.max)
                    nc.scalar.dma_start(pmax_d[:, :], pmax[0:1, 0:1])
                    nc.scalar.dma_start(degw_d[:, 0:off], degsb[:, 0:off])

            nc.scalar.dma_start(degw_d[:, send_end:], degsb[:, send_end:])

    nc.compile()
    return nc


# --------------------------------------------------------------------------
# launch B: out^T = WA^T X^T + WB^T (sX)^T + b, fp16/fp8, transposed layout
# --------------------------------------------------------------------------

def _build_main_program(srep_fp8=False):
    nc = bacc.Bacc("TRN2", target_bir_lowering=False, debug=False,
                   num_devices=NCORES)
    A = mybir.AluOpType
    X = mybir.AxisListType.X
    sdt = fp8 if srep_fp8 else fp16

    xt_d = nc.dram_tensor("xt", [F, NPAD], fp16, kind="ExternalInput")
    wk_d = nc.dram_tensor("wk", [128, KCH * F], fp16, kind="ExternalInput")
    degw_d = nc.dram_tensor("degw", [128, 2 * T], fp16, kind="ExternalInput")
    aux_d = nc.dram_tensor("aux", [128, 8], f32, kind="ExternalInput")
    out_d = nc.dram_tensor("out", [F, NPAD], fp16, kind="ExternalOutput")

    XCH = 4                    # xt / srep / out DMA chunks
    CW = NPAD // XCH           # 3136
    GW = 448                   # matmul group width (PSUM bank = 512 f32 max)
    GPC = CW // GW             # 7 groups per chunk

    with tile.TileContext(nc) as tc:
        with (
            tc.tile_pool(name="const", bufs=1) as constp,
            tc.tile_pool(name="xt", bufs=1) as xtp,
            tc.tile_pool(name="outp", bufs=1) as outp,
            tc.tile_pool(name="small", bufs=1) as smallp,
            tc.tile_pool(name="sx", bufs=5) as sxp,
            tc.tile_pool(name="srepp", bufs=1) as srepp,
            tc.tile_pool(name="psf", bufs=7, space="PSUM") as psf,
            tc.tile_pool(name="pst", bufs=1, space="PSUM") as pst,
            tc.tile_pool(name="dram", bufs=1, space="DRAM") as dramp,
        ):
            # ---- tiny input DMAs first ON THE SYNC RING so their packets
            # drain ahead of the xt flood (Q10 small DMAs crawl behind Q1) ---
            with tc.high_priority():
                degw_sb = smallp.tile([128, 2 * T], fp16)
                nc.sync.dma_start(degw_sb[:, :], degw_d[:, :])
                aux_sb = smallp.tile([128, 8], f32)
                nc.sync.dma_start(aux_sb[:, :], aux_d[:, :])
                # ACT table pre-warm with a dependency-free input
                warm_in = smallp.tile([1, 1], f32)
                nc.vector.memset(warm_in[:, :], 0.0)
                nc.scalar.activation(warm_in[:, :], warm_in[:, :],
                                     mybir.ActivationFunctionType.Identity,
                                     bias=0.0, scale=1.0)
                ident16 = smallp.tile([128, 128], fp16)
                masks.make_identity(nc, ident16[:, :])
                # PE HAM warm-up: ~3.4us of dependency-free matmuls so the
                # real stream starts at 2.4 GHz
                ps_w = psf.tile([128, GW], f32, tag="psf")
                for _ in range(16):
                    nc.tensor.matmul(ps_w[:, 0:128], ident16[:, :], ident16[:, :],
                                     start=True, stop=True)
            wk_sb = constp.tile([128, KCH * F], fp16)
            nc.scalar.dma_start(wk_sb[:, :], wk_d[:, :])

            # ---- node features (sync queue, early) -------------------------
            xt_sb = []
            for c in range(XCH):
                xt_c = xtp.tile([128, CW], fp16, name=f"xt{c}")
                nc.sync.dma_start(xt_c[:, :], xt_d[:, c * CW : (c + 1) * CW])
                xt_sb.append(xt_c)

            # ---- s = (deg - inw) * recip(m), transposed to node order ------
            with tc.high_priority():
                minv = smallp.tile([128, 1], f32)
                nc.vector.reciprocal(minv[:, :], aux_sb[:, 0:1])
                s_sb = smallp.tile([128, T], f32)
                nc.vector.tensor_sub(s_sb[:, :], degw_sb[:, :T], degw_sb[:, T:])
                s16 = smallp.tile([128, 128], fp16)
                nc.vector.memset(s16[:, :], 0.0)
                nc.vector.tensor_scalar_mul(s16[:, 0:T], s_sb[:, :], minv[:, 0:1])
            with tc.high_priority():
                ps_t = pst.tile([128, 128], fp16, tag="pst")
                nc.tensor.transpose(ps_t[:, :], s16[:, :], ident16[:, :])
                s_tr = smallp.tile([128, 128], sdt)
                nc.vector.tensor_copy(s_tr[:, :], ps_t[:, :])
                strow_d = dramp.tile([T, 128], sdt)
                nc.scalar.dma_start(strow_d[:, :], s_tr[0:T, :])

            # ---- bias column ----------------------------------------------
            with tc.high_priority():
                btot_col = smallp.tile([128, 1], f32)
                nc.vector.tensor_reduce(btot_col[:, :], aux_sb[:, 1:8],
                                        axis=X, op=A.add)

            # ---- weights: WA | WB in fp16 ----------------------------------
            # WA = W0 - W2 - 2W3 - 3W4 - 4W5,  WB = W1 + 2W2 + 3W3 + 4W4 + 5W5
            def wkk(k):
                return wk_sb[:, k * F : (k + 1) * F]
            wa16 = constp.tile([128, F], fp16)
            wb16 = constp.tile([128, F], fp16)
            nc.vector.scalar_tensor_tensor(wa16[:, :], wkk(2), -1.0, wkk(0), op0=A.mult, op1=A.add)
            nc.vector.scalar_tensor_tensor(wa16[:, :], wkk(3), -2.0, wa16[:, :], op0=A.mult, op1=A.add)
            nc.vector.scalar_tensor_tensor(wa16[:, :], wkk(4), -3.0, wa16[:, :], op0=A.mult, op1=A.add)
            nc.vector.scalar_tensor_tensor(wa16[:, :], wkk(5), -4.0, wa16[:, :], op0=A.mult, op1=A.add)
            nc.vector.scalar_tensor_tensor(wb16[:, :], wkk(2), 2.0, wkk(1), op0=A.mult, op1=A.add)
            nc.vector.scalar_tensor_tensor(wb16[:, :], wkk(3), 3.0, wb16[:, :], op0=A.mult, op1=A.add)
            nc.vector.scalar_tensor_tensor(wb16[:, :], wkk(4), 4.0, wb16[:, :], op0=A.mult, op1=A.add)
            nc.vector.scalar_tensor_tensor(wb16[:, :], wkk(5), 5.0, wb16[:, :], op0=A.mult, op1=A.add)

            # ---- s broadcast: every partition re-reads the s row -----------
            sflat = strow_d[:, :]
            srep_sb = []
            for c in range(XCH):
                srep_c = srepp.tile([128, CW], sdt, name=f"srep{c}")
                srcap = bass.AP(sflat.tensor, sflat.offset + c * CW, [[0, 128], [1, CW]])
                nc.scalar.dma_start(srep_c[:, :], srcap)
                srep_sb.append(srep_c)

            # ---- main loop -------------------------------------------------
            for c in range(XCH):
                out_c = outp.tile([128, CW], fp16, name=f"out{c}")
                srep_c = srep_sb[c]
                sxs, psFs = [], []
                for g in range(GPC):
                    n0 = g * GW
                    sx = sxp.tile([128, GW], fp16, tag="sx")
                    # split the s*x multiply between DVE and the idle GpSimd
                    eng = nc.gpsimd if (c * GPC + g) % 3 == 2 else nc.vector
                    eng.tensor_tensor(sx[:, :], xt_sb[c][:, n0 : n0 + GW],
                                      srep_c[:, n0 : n0 + GW], op=A.mult)
                    sxs.append(sx)
                for g in range(GPC):
                    n0 = g * GW
                    psF = psf.tile([128, GW], f32, tag="psf")
                    nc.tensor.matmul(psF[:, :], wa16[:, :],
                                     xt_sb[c][:, n0 : n0 + GW], start=True, stop=False)
                    psFs.append(psF)
                for g in range(GPC):
                    nc.tensor.matmul(psFs[g][:, :], wb16[:, :], sxs[g][:, :],
                                     start=False, stop=True)
                for g in range(GPC):
                    n0 = g * GW
                    nc.scalar.activation(out_c[:, n0 : n0 + GW], psFs[g][:, :],
                                         mybir.ActivationFunctionType.Identity,
                                         bias=btot_col[:, 0:1], scale=1.0)
                nc.sync.dma_start(out_d[:, c * CW : (c + 1) * CW], out_c[:, :])

    nc.compile()
    return nc


# --------------------------------------------------------------------------
# host driver
# --------------------------------------------------------------------------

def kernel(nodes, edges, senders, receivers, Wk, bk, bias):
    nodes = np.ascontiguousarray(np.asarray(nodes, np.float32))
    edges = np.ascontiguousarray(np.asarray(edges, np.float32))
    senders = np.asarray(senders)
    receivers = np.asarray(receivers)
    Wk = np.ascontiguousarray(np.asarray(Wk, np.float32))
    bk = np.asarray(bk, np.float32)
    bias = np.asarray(bias, np.float32)
    assert nodes.shape == (N_NODES, F) and Wk.shape == (KCH, F, F)

    cores = list(range(NCORES))

    # ---- common packing layout across cores (permutation + zero fill only) -
    lnS_c, lnR_c, wS_c, wR_c, clsS_c, clsR_c = [], [], [], [], [], []
    for c in cores:
        mS = (senders // NPC) == c
        mR = (receivers // NPC) == c
        lnS = (senders[mS] - c * NPC).astype(np.int64)
        lnR = (receivers[mR] - c * NPC).astype(np.int64)
        lnS_c.append(lnS); lnR_c.append(lnR)
        wS_c.append(edges[mS]); wR_c.append(edges[mR])
        for ln, dst in ((lnS, clsS_c), (lnR, clsR_c)):
            cnt = np.bincount(ln, minlength=NPC)
            dst.append(np.maximum(((np.maximum(cnt, 1) + 7) // 8) * 8, 8))
    planS = _CommonPlan(clsS_c)
    planR = _CommonPlan(clsR_c)

    ncls = len(planS.classes) + len(planR.classes)
    MASKW = 32 * ncls
    in_a = []
    evmaps = []  # (partS, colS, partR, colR)
    maskblk = np.zeros((128, MASKW), np_fp8)
    mi = 0
    for plan in (planS, planR):
        for cl in plan.classes:
            k = 128 // cl
            for i in range(k):
                maskblk[i * cl : (i + 1) * cl, mi * 32 + i] = 1.0
            mi += 1
    for c in cores:
        data = np.zeros((128, MASKW + planS.ncols + planR.ncols), np_fp8)
        data[:, :MASKW] = maskblk
        pS, cS = planS.fill(data, MASKW, lnS_c[c], wS_c[c], NPC)
        pR, cR = planR.fill(data, MASKW + planS.ncols, lnR_c[c], wR_c[c], NPC)
        evmaps.append((pS, cS, pR, cR))
        in_a.append({"ed": np.ascontiguousarray(data)})

    key = ("edge", tuple(planS.classes), tuple(planR.classes),
           planS.ncols, planR.ncols,
           tuple((W, tuple(map(tuple, g))) for W, g in planS.chunks),
           tuple((W, tuple(map(tuple, g))) for W, g in planR.chunks))
    if key not in _prog_cache:
        _prog_cache[key] = _build_edge_program(planS, planR)
    ncA = _prog_cache[key]

    res_a = run_bass_kernel_spmd(ncA, in_a, cores, trace=TRACE)

    # combine the 8 device partial maxima (selection, no arithmetic)
    m = max(float(res_a.results[c]["pmax"][0, 0]) for c in cores)

    # ---- host permutes packed sums into node order (selection only) -------
    if ("main",) not in _prog_cache:
        _prog_cache[("main",)] = _build_main_program()
    ncB = _prog_cache[("main",)]

    bkvec = np.concatenate([bk, bias.reshape(1, F)], axis=0)  # [7, F]
    wk16 = np.ascontiguousarray(
        Wk.transpose(1, 0, 2).reshape(128, KCH * F).astype(np.float16))
    in_b = []
    for c in cores:
        pS, cS, pR, cR = evmaps[c]
        dw = res_a.results[c]["degw"]                 # [128, EVC] fp16
        degw = np.zeros((128, 2 * T), np.float16)
        node = np.arange(NPC)
        degw[node % 128, node // 128] = dw[pS, cS]
        degw[node % 128, T + node // 128] = dw[pR, cR + planS.evac_cols]
        aux = np.zeros((128, 8), np.float32)
        aux[:, 0] = m
        aux[:, 1:8] = bkvec.T                          # [128(fo), 7]
        xt = np.zeros((F, NPAD), np.float16)
        xt[:, :NPC] = nodes[c * NPC : (c + 1) * NPC].T
        in_b.append({"xt": xt, "wk": wk16, "degw": degw, "aux": aux})
    res_b = run_bass_kernel_spmd(ncB, in_b, cores, trace=TRACE)

    ta = res_a.exec_time_ns
    tb = res_b.exec_time_ns
    LAST["exec_a_ns"] = ta
    LAST["exec_b_ns"] = tb
    LAST["exec_time_ns"] = (ta + tb) if (ta is not None and tb is not None) else None

    out = np.empty((N_NODES, F), np.float32)
    for c in cores:
        o = res_b.results[c]["out"]
        out[c * NPC : (c + 1) * NPC] = o.astype(np.float32).T[:NPC]
    return out


# revision 21
# speedup vs baseline: 1.0898x; 1.0898x over previous
"""ChebConv (K=6) message-passing kernel for 8 Trainium2 NeuronCores.

Math: the reference's GraphNetwork pass multiplies each node's features by a
per-node scalar s = (deg - in_w) / max(deg) (deg = segment_sum(edges, senders),
in_w = segment_sum(edges, receivers)), and the Chebyshev recurrence
Tx_k = 2*Tx_{k-1} - Tx_{k-2} stays rank-1 per node: Tx_k = (1 + k*(s-1)) * x.
Hence
    out = X @ WA + s * (X @ WB) + b_tot
with WA = sum_k (1-k) Wk[k], WB = sum_k k Wk[k], b_tot = sum_k bk[k] + bias.

Sharding: nodes are block-sharded over 8 cores (12500 each, padded to 12544).
Edges are routed on the host (index permutation + zero fill only, no float
arithmetic) to the core owning their sender (for deg) / receiver (for in_w).

Two launches (an in-kernel AllReduce costs ~60us of comm-subsystem init, far
more than a second NEFF):
  A: edge kernel -- segment sums as PE mask-matmuls.  Edges are packed on the
     host into fp8 columns of 128 slots; nodes are degree-classed (slot count
     rounded to a multiple of 8) so a column holds k = 128//class nodes at
     fixed offsets.  One [128,32] 0/1 mask per class turns a 512-column chunk
     into per-node sums via a single matmul per PSUM quadrant
     (tile_position col-tiling).  DVE max-reduce + gpsimd partition reduce
     give the per-core max(deg).
  host: m = max of the 8 device partial maxima (selection only); deg/in_w
     values are host-permuted (selection) from packed order to node order.
  B: main kernel -- s = (deg-inw)*recip(m); transposed layout outT[fo, n]:
     psF = WA^T @ X + WB^T @ (s*X) accumulated in PSUM, evacuated by ACT with
     per-partition bias; s broadcast along partitions via an fp8 DRAM
     broadcast read (srep).
"""

import sys

sys.path.insert(0, "/opt/trn_rl_repo")

import numpy as np
import ml_dtypes

import concourse.bacc as bacc
import concourse.bass as bass
import concourse.mybir as mybir
import concourse.tile as tile
from concourse import bass_isa, masks
from concourse.bass_utils import run_bass_kernel_spmd

N_NODES = 100000
F = 128
KCH = 6
NCORES = 8
NPC = N_NODES // NCORES       # 12500 nodes per core
T = (NPC + 127) // 128        # 98 node tiles per core
NPAD = T * 128                # 12544 (cols 12500.. are zero padding)

f32 = mybir.dt.float32
bf16 = mybir.dt.bfloat16
fp16 = mybir.dt.float16
fp8 = mybir.dt.float8e4

np_fp8 = ml_dtypes.float8_e4m3fn

# test.py knobs (harness never touches these)
TRACE = False
LAST = {}

PW = 512                      # PSUM chunk width (one bank of f32)

_prog_cache = {}


# --------------------------------------------------------------------------
# host-side edge packing for the PE segment-sum (permutation + zero-fill only)
# --------------------------------------------------------------------------

class _CommonPlan:
    """Shared (all-cores) packing layout for one buffer (sender- or
    receiver-keyed edges).

    Column layout: a column has 128 slots; it holds k = 128//cls nodes of the
    same degree-class cls (cls = ceil8(cnt)), node i at slot rows
    [i*cls, i*cls + cnt).  Per-class column counts are the max over cores so
    one program serves all cores (unused columns stay zero).  Columns are
    grouped into pieces of <= PW columns (one matmul each), pieces into
    chunks of <= 4 (PSUM quadrants).
    """

    def __init__(self, cls_per_core):
        classes = set()
        for cls in cls_per_core:
            classes.update(int(c) for c in np.unique(cls))
        self.classes = sorted(classes)
        self.colbase_cl = []       # virtual (class-contiguous) column base
        vcol0 = 0
        pieces = []  # [cls_idx, vcol0, ncols]
        for ci, cl in enumerate(self.classes):
            k = 128 // cl
            n = max(int((cls == cl).sum()) for cls in cls_per_core)
            ncol = (n + k - 1) // k
            self.colbase_cl.append(vcol0)
            p0 = 0
            while p0 < ncol:
                w = min(PW, ncol - p0)
                pieces.append([ci, vcol0 + p0, w])
                p0 += w
            vcol0 += ncol
        self.nvcols = vcol0
        # chunks: group pieces (desc by width) into groups of 4; every piece
        # occupies a full chunk-width data stripe (zero-padded) so the matmul
        # covers the whole PSUM quadrant (stale-PSUM-free max/evac).
        pieces.sort(key=lambda p: -p[2])
        self.chunks = []   # list of (W, [(cls_idx, padded_base, vcol0, w), ...])
        ncols = 0
        for i in range(0, len(pieces), 4):
            grp = pieces[i : i + 4]
            W = grp[0][2]
            g2 = []
            for ci, v0, w in grp:
                g2.append((ci, ncols, v0, w))
                ncols += W
            self.chunks.append((W, g2))
        self.ncols = ncols
        self.evac_cols = sum(W for W, _ in self.chunks)
        # piece lookup tables for vcol -> padded data col
        self._pv0 = np.array([p[2] for _, g in self.chunks for p in g], np.int64)
        self._ppb = np.array([p[1] for _, g in self.chunks for p in g], np.int64)

    def _v2d(self, vcol):
        """Map virtual cols to padded data cols (vectorized)."""
        # piece index: each piece covers [v0, v0+w) and pieces are disjoint
        order = np.argsort(self._pv0)
        v0s = self._pv0[order]
        pbs = self._ppb[order]
        pi = np.searchsorted(v0s, vcol, side="right") - 1
        return pbs[pi] + (vcol - v0s[pi])

    def fill(self, data, colbase, ln, vals, NPCL, cls):
        """Scatter edge values into data[:, colbase:...]; returns per-node
        (evac_partition, evac_col) for reading packed sums back."""
        cnt = np.bincount(ln, minlength=NPCL)
        assert cnt.max() <= 128, f"node degree {cnt.max()} > 128 unsupported"
        node_vcol = np.zeros(NPCL, np.int64)
        node_row = np.zeros(NPCL, np.int64)
        for ci, cl in enumerate(self.classes):
            nodes = np.nonzero(cls == cl)[0]
            if not len(nodes):
                continue
            k = 128 // cl
            j = np.arange(len(nodes))
            node_vcol[nodes] = self.colbase_cl[ci] + j // k
            node_row[nodes] = (j % k) * cl
        node_dcol = self._v2d(node_vcol)
        order = np.argsort(ln, kind="stable")
        se = ln[order]
        sv = vals[order]
        first = np.concatenate(([0], np.cumsum(cnt)[:-1]))
        slot = np.arange(len(se), dtype=np.int64) - first[se]
        data[node_row[se] + slot, colbase + node_dcol[se]] = sv.astype(data.dtype)
        # evac mapping: chunk occupies evac cols [off, off+W); node value at
        # partition 32*quad + slot_idx, col off + (vcol - piece_vcol0)
        part = np.zeros(NPCL, np.int64)
        col = np.zeros(NPCL, np.int64)
        off = 0
        for W, grp in self.chunks:
            for quad, (ci, pb, v0, w) in enumerate(grp):
                cl = self.classes[ci]
                sel = np.nonzero((cls == cl) & (node_vcol >= v0) & (node_vcol < v0 + w))[0]
                part[sel] = 32 * quad + node_row[sel] // cl
                col[sel] = off + node_vcol[sel] - v0
            off += W
        return part, col


def _build_edge_program(plan):
    """Launch A: t = deg - in_w per node via mask matmuls, plus max(deg).

    Both edge buffers use the SAME packing plan, so in_w accumulates into the
    deg PSUM with negated masks (subtraction for free); a DVE max-reduce reads
    deg between the two matmul sets."""
    nc = bacc.Bacc("TRN2", target_bir_lowering=False, debug=False,
                   num_devices=NCORES)
    A = mybir.AluOpType
    X = mybir.AxisListType.X

    ncls = len(plan.classes)
    MASKW = 64 * ncls            # [pos | neg] 32-wide mask pair per class
    NCOLS = plan.ncols
    EVC = plan.evac_cols
    nchunks = len(plan.chunks)

    ed_d = nc.dram_tensor("ed", [128, MASKW + 2 * NCOLS], fp8, kind="ExternalInput")
    degw_d = nc.dram_tensor("degw", [128, EVC], fp16, kind="ExternalOutput")
    pmax_d = nc.dram_tensor("pmax", [1, 1], f32, kind="ExternalOutput")

    with tile.TileContext(nc) as tc:
        with (
            tc.tile_pool(name="ed", bufs=1) as edp,
            tc.tile_pool(name="small", bufs=1) as smallp,
            tc.tile_pool(name="ps", bufs=min(nchunks, 6), space="PSUM") as psp,
        ):
            ed_sb = edp.tile([128, MASKW + 2 * NCOLS], fp8)
            nc.sync.dma_start(ed_sb[:, :MASKW], ed_d[:, :MASKW])
            # the two value regions split into 2 DMAs each for pipelining
            for base in (MASKW, MASKW + NCOLS):
                h = NCOLS // 2
                nc.sync.dma_start(ed_sb[:, base : base + h], ed_d[:, base : base + h])
                nc.sync.dma_start(ed_sb[:, base + h : base + NCOLS],
                                  ed_d[:, base + h : base + NCOLS])

            degsb = smallp.tile([128, EVC], fp16)
            dmax = smallp.tile([128, nchunks], f32)
            nc.vector.memset(dmax[:, :], 0.0)

            off = 0
            for kc, (W, grp) in enumerate(plan.chunks):
                ps = psp.tile([128, PW], f32, tag="ps")
                nq = len(grp)
                for quad, (ci, pb, v0, w) in enumerate(grp):
                    nc.tensor.matmul(
                        ps[32 * quad : 32 * quad + 32, 0:W],
                        ed_sb[:, ci * 64 : ci * 64 + 32],
                        ed_sb[:, MASKW + pb : MASKW + pb + W],
                        start=True, stop=True,
                        tile_position=(0, 32 * quad),
                    )
                nc.vector.tensor_reduce(dmax[0 : 32 * nq, kc : kc + 1],
                                        ps[0 : 32 * nq, 0:W], axis=X, op=A.max)
                for quad, (ci, pb, v0, w) in enumerate(grp):
                    nc.tensor.matmul(
                        ps[32 * quad : 32 * quad + 32, 0:W],
                        ed_sb[:, ci * 64 + 32 : ci * 64 + 64],
                        ed_sb[:, MASKW + NCOLS + pb : MASKW + NCOLS + pb + W],
                        start=False, stop=True, skip_group_check=True,
                        tile_position=(0, 32 * quad),
                    )
                # alternate evacuation engine so neither DVE nor ACT binds
                if kc % 2 == 0:
                    nc.scalar.activation(degsb[0 : 32 * nq, off : off + W],
                                         ps[0 : 32 * nq, 0:W],
                                         mybir.ActivationFunctionType.Identity,
                                         bias=0.0, scale=1.0)
                else:
                    nc.vector.tensor_copy(degsb[0 : 32 * nq, off : off + W],
                                          ps[0 : 32 * nq, 0:W])
                off += W

            gmax = smallp.tile([128, 1], f32)
            nc.vector.tensor_reduce(gmax[:, :], dmax[:, :], axis=X, op=A.max)
            pmax = smallp.tile([128, 1], f32)
            nc.gpsimd.partition_all_reduce(pmax[:, :], gmax[:, :], channels=128,
                                           reduce_op=bass_isa.ReduceOp.max)
            nc.scalar.dma_start(pmax_d[:, :], pmax[0:1, 0:1])
            nc.scalar.dma_start(degw_d[:, :], degsb[:, :])

    nc.compile()
    return nc


# --------------------------------------------------------------------------
# launch B: out^T = WA^T X^T + WB^T (sX)^T + b, fp16/fp8, transposed layout
# --------------------------------------------------------------------------

def _build_main_program():
    """Launch B: outT = WA^T X + (WB/m)^T (t*X) + b.  The per-node factor t
    arrives pre-broadcast from the host (trep, pure replication), so the
    whole kernel is a straight-line DMA/compute pipeline with no on-device
    transpose or DRAM roundtrip."""
    nc = bacc.Bacc("TRN2", target_bir_lowering=False, debug=False,
                   num_devices=NCORES)
    A = mybir.AluOpType
    X = mybir.AxisListType.X

    xt_d = nc.dram_tensor("xt", [F, NPAD], fp16, kind="ExternalInput")
    wk_d = nc.dram_tensor("wk", [128, KCH * F], fp16, kind="ExternalInput")
    trep_d = nc.dram_tensor("trep", [F, NPAD], fp16, kind="ExternalInput")
    aux_d = nc.dram_tensor("aux", [128, 8], f32, kind="ExternalInput")
    out_d = nc.dram_tensor("out", [F, NPAD], fp16, kind="ExternalOutput")

    XCH = 7                    # xt / trep / out DMA chunks
    CW = NPAD // XCH           # 1792
    GW = 448                   # matmul group width (PSUM bank = 512 f32 max)
    GPC = CW // GW             # 4 groups per chunk

    with tile.TileContext(nc) as tc:
        with (
            tc.tile_pool(name="const", bufs=1) as constp,
            tc.tile_pool(name="xt", bufs=1) as xtp,
            tc.tile_pool(name="trepp", bufs=1) as trepp,
            tc.tile_pool(name="outp", bufs=1) as outp,
            tc.tile_pool(name="small", bufs=1) as smallp,
            tc.tile_pool(name="sx", bufs=6) as sxp,
            tc.tile_pool(name="psf", bufs=8, space="PSUM") as psf,
        ):
            with tc.high_priority():
                aux_sb = smallp.tile([128, 8], f32)
                nc.sync.dma_start(aux_sb[:, :], aux_d[:, :])
                # ACT table pre-warm with a dependency-free input
                warm_in = smallp.tile([1, 1], f32)
                nc.vector.memset(warm_in[:, :], 0.0)
                nc.scalar.activation(warm_in[:, :], warm_in[:, :],
                                     mybir.ActivationFunctionType.Identity,
                                     bias=0.0, scale=1.0)
                ident16 = smallp.tile([128, 128], fp16)
                masks.make_identity(nc, ident16[:, :])
                # PE HAM warm-up: ~3.4us of dependency-free matmuls so the
                # real stream starts at 2.4 GHz
                ps_w = psf.tile([128, GW], f32, tag="psf")
                for _ in range(16):
                    nc.tensor.matmul(ps_w[:, 0:128], ident16[:, :], ident16[:, :],
                                     start=True, stop=True)
            wk_sb = constp.tile([128, KCH * F], fp16)
            nc.scalar.dma_start(wk_sb[:, :], wk_d[:, :])

            # ---- node features + t broadcast, interleaved on the sync ring -
            xt_sb, trep_sb = [], []
            for c in range(XCH):
                xt_c = xtp.tile([128, CW], fp16, name=f"xt{c}")
                nc.sync.dma_start(xt_c[:, :], xt_d[:, c * CW : (c + 1) * CW])
                xt_sb.append(xt_c)
                trep_c = trepp.tile([128, CW], fp16, name=f"trep{c}")
                nc.sync.dma_start(trep_c[:, :], trep_d[:, c * CW : (c + 1) * CW])
                trep_sb.append(trep_c)

            with tc.high_priority():
                minv = smallp.tile([128, 1], f32)
                nc.vector.reciprocal(minv[:, :], aux_sb[:, 0:1])
                btot_col = smallp.tile([128, 1], f32)
                nc.vector.tensor_reduce(btot_col[:, :], aux_sb[:, 1:8],
                                        axis=X, op=A.add)

            # ---- weights: WA | WB/m in fp16 --------------------------------
            # WA = W0 - W2 - 2W3 - 3W4 - 4W5,  WB = W1 + 2W2 + 3W3 + 4W4 + 5W5
            def wkk(k):
                return wk_sb[:, k * F : (k + 1) * F]
            wa16 = constp.tile([128, F], fp16)
            wb16 = constp.tile([128, F], fp16)
            nc.vector.scalar_tensor_tensor(wa16[:, :], wkk(2), -1.0, wkk(0), op0=A.mult, op1=A.add)
            nc.vector.scalar_tensor_tensor(wa16[:, :], wkk(3), -2.0, wa16[:, :], op0=A.mult, op1=A.add)
            nc.vector.scalar_tensor_tensor(wa16[:, :], wkk(4), -3.0, wa16[:, :], op0=A.mult, op1=A.add)
            nc.vector.scalar_tensor_tensor(wa16[:, :], wkk(5), -4.0, wa16[:, :], op0=A.mult, op1=A.add)
            nc.vector.scalar_tensor_tensor(wb16[:, :], wkk(2), 2.0, wkk(1), op0=A.mult, op1=A.add)
            nc.vector.scalar_tensor_tensor(wb16[:, :], wkk(3), 3.0, wb16[:, :], op0=A.mult, op1=A.add)
            nc.vector.scalar_tensor_tensor(wb16[:, :], wkk(4), 4.0, wb16[:, :], op0=A.mult, op1=A.add)
            nc.vector.scalar_tensor_tensor(wb16[:, :], wkk(5), 5.0, wb16[:, :], op0=A.mult, op1=A.add)
            nc.vector.tensor_scalar_mul(wb16[:, :], wb16[:, :], minv[:, 0:1])

            # ---- main loop -------------------------------------------------
            for c in range(XCH):
                out_c = outp.tile([128, CW], fp16, name=f"out{c}")
                sxs, psFs = [], []
                for g in range(GPC):
                    n0 = g * GW
                    sx = sxp.tile([128, GW], fp16, tag="sx")
                    nc.vector.tensor_tensor(sx[:, :], xt_sb[c][:, n0 : n0 + GW],
                                            trep_sb[c][:, n0 : n0 + GW], op=A.mult)
                    sxs.append(sx)
                for g in range(GPC):
                    n0 = g * GW
                    psF = psf.tile([128, GW], f32, tag="psf")
                    nc.tensor.matmul(psF[:, :], wa16[:, :],
                                     xt_sb[c][:, n0 : n0 + GW], start=True, stop=False)
                    psFs.append(psF)
                for g in range(GPC):
                    nc.tensor.matmul(psFs[g][:, :], wb16[:, :], sxs[g][:, :],
                                     start=False, stop=True)
                for g in range(GPC):
                    n0 = g * GW
                    # split evacuation between ACT and DVE
                    if (c * GPC + g) % 3 == 2:
                        nc.vector.tensor_scalar_add(out_c[:, n0 : n0 + GW],
                                                    psFs[g][:, :], btot_col[:, 0:1])
                    else:
                        nc.scalar.activation(out_c[:, n0 : n0 + GW], psFs[g][:, :],
                                             mybir.ActivationFunctionType.Identity,
                                             bias=btot_col[:, 0:1], scale=1.0)
                nc.scalar.dma_start(out_d[:, c * CW : (c + 1) * CW], out_c[:, :])

    nc.compile()
    return nc


# --------------------------------------------------------------------------
# host driver
# --------------------------------------------------------------------------

def kernel(nodes, edges, senders, receivers, Wk, bk, bias):
    nodes = np.ascontiguousarray(np.asarray(nodes, np.float32))
    edges = np.ascontiguousarray(np.asarray(edges, np.float32))
    senders = np.asarray(senders)
    receivers = np.asarray(receivers)
    Wk = np.ascontiguousarray(np.asarray(Wk, np.float32))
    bk = np.asarray(bk, np.float32)
    bias = np.asarray(bias, np.float32)
    assert nodes.shape == (N_NODES, F) and Wk.shape == (KCH, F, F)

    cores = list(range(NCORES))

    # ---- common packing layout across cores AND both buffers ---------------
    # (sender and receiver edges of a node share one slot assignment so the
    # device can accumulate in_w with negated masks: permutation + zero fill)
    lnS_c, lnR_c, wS_c, wR_c, cls_c = [], [], [], [], []
    for c in cores:
        mS = (senders // NPC) == c
        mR = (receivers // NPC) == c
        lnS = (senders[mS] - c * NPC).astype(np.int64)
        lnR = (receivers[mR] - c * NPC).astype(np.int64)
        lnS_c.append(lnS); lnR_c.append(lnR)
        wS_c.append(edges[mS]); wR_c.append(edges[mR])
        cnt = np.maximum(np.bincount(lnS, minlength=NPC),
                         np.bincount(lnR, minlength=NPC))
        cls_c.append(np.maximum(((np.maximum(cnt, 1) + 7) // 8) * 8, 8))
    plan = _CommonPlan(cls_c)

    ncls = len(plan.classes)
    MASKW = 64 * ncls
    maskblk = np.zeros((128, MASKW), np_fp8)
    for ci, cl in enumerate(plan.classes):
        k = 128 // cl
        for i in range(k):
            maskblk[i * cl : (i + 1) * cl, ci * 64 + i] = 1.0
            maskblk[i * cl : (i + 1) * cl, ci * 64 + 32 + i] = -1.0
    in_a = []
    evmaps = []
    for c in cores:
        data = np.zeros((128, MASKW + 2 * plan.ncols), np_fp8)
        data[:, :MASKW] = maskblk
        pS, cS = plan.fill(data, MASKW, lnS_c[c], wS_c[c], NPC, cls_c[c])
        pR, cR = plan.fill(data, MASKW + plan.ncols, lnR_c[c], wR_c[c], NPC, cls_c[c])
        assert np.array_equal(pS, pR) and np.array_equal(cS, cR)
        evmaps.append((pS, cS))
        in_a.append({"ed": np.ascontiguousarray(data)})

    key = ("edge", tuple(plan.classes), plan.ncols,
           tuple((W, tuple(map(tuple, g))) for W, g in plan.chunks))
    if key not in _prog_cache:
        _prog_cache[key] = _build_edge_program(plan)
    ncA = _prog_cache[key]

    res_a = run_bass_kernel_spmd(ncA, in_a, cores, trace=TRACE)

    # combine the 8 device partial maxima (selection, no arithmetic)
    m = max(float(res_a.results[c]["pmax"][0, 0]) for c in cores)

    # ---- host permutes packed sums into node order (selection only) -------
    if ("main",) not in _prog_cache:
        _prog_cache[("main",)] = _build_main_program()
    ncB = _prog_cache[("main",)]

    bkvec = np.concatenate([bk, bias.reshape(1, F)], axis=0)  # [7, F]
    wk16 = np.ascontiguousarray(
        Wk.transpose(1, 0, 2).reshape(128, KCH * F).astype(np.float16))
    in_b = []
    for c in cores:
        pS, cS = evmaps[c]
        dw = res_a.results[c]["degw"]                 # [128, EVC] fp16 (= t)
        trow = np.zeros(NPAD, np.float16)
        trow[:NPC] = dw[pS, cS]                       # node order (selection)
        trep = np.ascontiguousarray(np.broadcast_to(trow[None, :], (F, NPAD)))
        aux = np.zeros((128, 8), np.float32)
        aux[:, 0] = m
        aux[:, 1:8] = bkvec.T                          # [128(fo), 7]
        xt = np.zeros((F, NPAD), np.float16)
        xt[:, :NPC] = nodes[c * NPC : (c + 1) * NPC].T
        in_b.append({"xt": xt, "wk": wk16, "trep": trep, "aux": aux})
    res_b = run_bass_kernel_spmd(ncB, in_b, cores, trace=TRACE)

    ta = res_a.exec_time_ns
    tb = res_b.exec_time_ns
    LAST["exec_a_ns"] = ta
    LAST["exec_b_ns"] = tb
    LAST["exec_time_ns"] = (ta + tb) if (ta is not None and tb is not None) else None

    out = np.empty((N_NODES, F), np.float32)
    for c in cores:
        o = res_b.results[c]["out"]
        out[c * NPC : (c + 1) * NPC] = o.astype(np.float32).T[:NPC]
    return out


# revision 28
# speedup vs baseline: 1.1226x; 1.0301x over previous
"""ChebConv (K=6) message-passing kernel for 8 Trainium2 NeuronCores.

Math: the reference's GraphNetwork pass multiplies each node's features by a
per-node scalar s = (deg - in_w) / max(deg) (deg = segment_sum(edges, senders),
in_w = segment_sum(edges, receivers)), and the Chebyshev recurrence
Tx_k = 2*Tx_{k-1} - Tx_{k-2} stays rank-1 per node: Tx_k = (1 + k*(s-1)) * x.
Hence
    out = X @ WA + s * (X @ WB) + b_tot
with WA = sum_k (1-k) Wk[k], WB = sum_k k Wk[k], b_tot = sum_k bk[k] + bias.

Sharding: nodes are block-sharded over 8 cores (12500 each, padded to 12544).
Edges are routed on the host (index permutation + zero fill only, no float
arithmetic) to the core owning their sender (for deg) / receiver (for in_w).

Two launches (an in-kernel AllReduce costs ~60us of comm-subsystem init, far
more than a second NEFF):
  A: edge kernel -- segment sums as PE mask-matmuls.  Edges are packed on the
     host into fp8 columns of 128 slots; nodes are degree-classed (slot count
     rounded to a multiple of 8) so a column holds k = 128//class nodes at
     fixed offsets.  One [128,32] 0/1 mask per class turns a 512-column chunk
     into per-node sums via a single matmul per PSUM quadrant
     (tile_position col-tiling).  DVE max-reduce + gpsimd partition reduce
     give the per-core max(deg).
  host: m = max of the 8 device partial maxima (selection only); deg/in_w
     values are host-permuted (selection) from packed order to node order.
  B: main kernel -- s = (deg-inw)*recip(m); transposed layout outT[fo, n]:
     psF = WA^T @ X + WB^T @ (s*X) accumulated in PSUM, evacuated by ACT with
     per-partition bias; s broadcast along partitions via an fp8 DRAM
     broadcast read (srep).
"""

import sys

sys.path.insert(0, "/opt/trn_rl_repo")

import numpy as np
import ml_dtypes

import concourse.bacc as bacc
import concourse.bass as bass
import concourse.mybir as mybir
import concourse.tile as tile
from concourse import bass_isa, masks
from concourse.bass_utils import run_bass_kernel_spmd

N_NODES = 100000
F = 128
KCH = 6
NCORES = 8
NPC = N_NODES // NCORES       # 12500 nodes per core
T = (NPC + 127) // 128        # 98 node tiles per core
NPAD = T * 128                # 12544 (cols 12500.. are zero padding)

f32 = mybir.dt.float32
bf16 = mybir.dt.bfloat16
fp16 = mybir.dt.float16
fp8 = mybir.dt.float8e4

np_fp8 = ml_dtypes.float8_e4m3fn

# test.py knobs (harness never touches these)
TRACE = False
LAST = {}

PW = 512                      # PSUM chunk width (one bank of f32)

_prog_cache = {}


# --------------------------------------------------------------------------
# host-side edge packing for the PE segment-sum (permutation + zero-fill only)
# --------------------------------------------------------------------------

class _CommonPlan:
    """Shared (all-cores) packing layout for one buffer (sender- or
    receiver-keyed edges).

    Column layout: a column has 128 slots; it holds k = 128//cls nodes of the
    same degree-class cls (cls = ceil8(cnt)), node i at slot rows
    [i*cls, i*cls + cnt).  Per-class column counts are the max over cores so
    one program serves all cores (unused columns stay zero).  Columns are
    grouped into pieces of <= PW columns (one matmul each), pieces into
    chunks of <= 4 (PSUM quadrants).
    """

    def __init__(self, cls_per_core):
        classes = set()
        for cls in cls_per_core:
            classes.update(int(c) for c in np.unique(cls))
        self.classes = sorted(classes)
        self.colbase_cl = []       # virtual (class-contiguous) column base
        vcol0 = 0
        pieces = []  # [cls_idx, vcol0, ncols]
        for ci, cl in enumerate(self.classes):
            k = 128 // cl
            n = max(int((cls == cl).sum()) for cls in cls_per_core)
            ncol = (n + k - 1) // k
            self.colbase_cl.append(vcol0)
            p0 = 0
            while p0 < ncol:
                w = min(PW, ncol - p0)
                pieces.append([ci, vcol0 + p0, w])
                p0 += w
            vcol0 += ncol
        self.nvcols = vcol0
        # chunks: group pieces (desc by width) into groups of 4; every piece
        # occupies a full chunk-width data stripe (zero-padded) so the matmul
        # covers the whole PSUM quadrant (stale-PSUM-free max/evac).
        pieces.sort(key=lambda p: -p[2])
        self.chunks = []   # list of (W, [(cls_idx, padded_base, vcol0, w), ...])
        ncols = 0
        for i in range(0, len(pieces), 4):
            grp = pieces[i : i + 4]
            W = grp[0][2]
            g2 = []
            for ci, v0, w in grp:
                g2.append((ci, ncols, v0, w))
                ncols += W
            self.chunks.append((W, g2))
        self.ncols = ncols
        self.evac_cols = sum(W for W, _ in self.chunks)
        # piece lookup tables for vcol -> padded data col
        self._pv0 = np.array([p[2] for _, g in self.chunks for p in g], np.int64)
        self._ppb = np.array([p[1] for _, g in self.chunks for p in g], np.int64)

    def _v2d(self, vcol):
        """Map virtual cols to padded data cols (vectorized)."""
        # piece index: each piece covers [v0, v0+w) and pieces are disjoint
        order = np.argsort(self._pv0)
        v0s = self._pv0[order]
        pbs = self._ppb[order]
        pi = np.searchsorted(v0s, vcol, side="right") - 1
        return pbs[pi] + (vcol - v0s[pi])

    def fill(self, data, colbase, ln, vals, NPCL, cls):
        """Scatter edge values into data[:, colbase:...]; returns per-node
        (evac_partition, evac_col) for reading packed sums back."""
        cnt = np.bincount(ln, minlength=NPCL)
        assert cnt.max() <= 128, f"node degree {cnt.max()} > 128 unsupported"
        node_vcol = np.zeros(NPCL, np.int64)
        node_row = np.zeros(NPCL, np.int64)
        for ci, cl in enumerate(self.classes):
            nodes = np.nonzero(cls == cl)[0]
            if not len(nodes):
                continue
            k = 128 // cl
            j = np.arange(len(nodes))
            node_vcol[nodes] = self.colbase_cl[ci] + j // k
            node_row[nodes] = (j % k) * cl
        node_dcol = self._v2d(node_vcol)
        order = np.argsort(ln, kind="stable")
        se = ln[order]
        sv = vals[order]
        first = np.concatenate(([0], np.cumsum(cnt)[:-1]))
        slot = np.arange(len(se), dtype=np.int64) - first[se]
        data[node_row[se] + slot, colbase + node_dcol[se]] = sv.astype(data.dtype)
        # evac mapping: chunk occupies evac cols [off, off+W); node value at
        # partition 32*quad + slot_idx, col off + (vcol - piece_vcol0)
        part = np.zeros(NPCL, np.int64)
        col = np.zeros(NPCL, np.int64)
        off = 0
        for W, grp in self.chunks:
            for quad, (ci, pb, v0, w) in enumerate(grp):
                cl = self.classes[ci]
                sel = np.nonzero((cls == cl) & (node_vcol >= v0) & (node_vcol < v0 + w))[0]
                part[sel] = 32 * quad + node_row[sel] // cl
                col[sel] = off + node_vcol[sel] - v0
            off += W
        return part, col


def _build_edge_program(plan):
    """Launch A: t = deg - in_w per node via mask matmuls, plus max(deg).

    Both edge buffers use the SAME packing plan, so in_w accumulates into the
    deg PSUM with negated masks (subtraction for free); a DVE max-reduce reads
    deg between the two matmul sets."""
    nc = bacc.Bacc("TRN2", target_bir_lowering=False, debug=False,
                   num_devices=NCORES)
    A = mybir.AluOpType
    X = mybir.AxisListType.X

    ncls = len(plan.classes)
    MASKW = 64 * ncls            # [pos | neg] 32-wide mask pair per class
    NCOLS = plan.ncols
    EVC = plan.evac_cols
    nchunks = len(plan.chunks)

    ed_d = nc.dram_tensor("ed", [128, MASKW + 2 * NCOLS], fp8, kind="ExternalInput")
    degw_d = nc.dram_tensor("degw", [128, EVC], fp16, kind="ExternalOutput")
    pmax_d = nc.dram_tensor("pmax", [1, 1], f32, kind="ExternalOutput")

    with tile.TileContext(nc) as tc:
        with (
            tc.tile_pool(name="ed", bufs=1) as edp,
            tc.tile_pool(name="small", bufs=1) as smallp,
            tc.tile_pool(name="ps", bufs=min(nchunks, 6), space="PSUM") as psp,
        ):
            ed_sb = edp.tile([128, MASKW + 2 * NCOLS], fp8)
            nc.sync.dma_start(ed_sb[:, :MASKW], ed_d[:, :MASKW])
            # value DMAs interleaved by chunk (pse_k, pre_k) so each chunk's
            # whole pipeline completes right after its own bytes land
            cb = 0
            for W, grp in plan.chunks:
                cw = sum(1 for _ in grp) * W
                for base in (MASKW, MASKW + NCOLS):
                    nc.sync.dma_start(ed_sb[:, base + cb : base + cb + cw],
                                      ed_d[:, base + cb : base + cb + cw])
                cb += cw

            degsb = smallp.tile([128, EVC], fp16)
            dmax = smallp.tile([128, nchunks], f32)
            nc.vector.memset(dmax[:, :], 0.0)

            off = 0
            for kc, (W, grp) in enumerate(plan.chunks):
                ps = psp.tile([128, PW], f32, tag="ps")
                nq = len(grp)
                for quad, (ci, pb, v0, w) in enumerate(grp):
                    nc.tensor.matmul(
                        ps[32 * quad : 32 * quad + 32, 0:W],
                        ed_sb[:, ci * 64 : ci * 64 + 32],
                        ed_sb[:, MASKW + pb : MASKW + pb + W],
                        start=True, stop=True,
                        tile_position=(0, 32 * quad),
                    )
                nc.vector.tensor_reduce(dmax[0 : 32 * nq, kc : kc + 1],
                                        ps[0 : 32 * nq, 0:W], axis=X, op=A.max)
                for quad, (ci, pb, v0, w) in enumerate(grp):
                    nc.tensor.matmul(
                        ps[32 * quad : 32 * quad + 32, 0:W],
                        ed_sb[:, ci * 64 + 32 : ci * 64 + 64],
                        ed_sb[:, MASKW + NCOLS + pb : MASKW + NCOLS + pb + W],
                        start=False, stop=True, skip_group_check=True,
                        tile_position=(0, 32 * quad),
                    )
                # alternate evacuation engine so neither DVE nor ACT binds
                if kc % 2 == 0:
                    nc.scalar.activation(degsb[0 : 32 * nq, off : off + W],
                                         ps[0 : 32 * nq, 0:W],
                                         mybir.ActivationFunctionType.Identity,
                                         bias=0.0, scale=1.0)
                else:
                    nc.vector.tensor_copy(degsb[0 : 32 * nq, off : off + W],
                                          ps[0 : 32 * nq, 0:W])
                off += W
                if kc == nchunks - 2:
                    # ship the finished head while the last chunk computes
                    nc.scalar.dma_start(degw_d[:, 0:off], degsb[:, 0:off])

            gmax = smallp.tile([128, 1], f32)
            nc.vector.tensor_reduce(gmax[:, :], dmax[:, :], axis=X, op=A.max)
            pmax = smallp.tile([128, 1], f32)
            nc.gpsimd.partition_all_reduce(pmax[:, :], gmax[:, :], channels=128,
                                           reduce_op=bass_isa.ReduceOp.max)
            nc.scalar.dma_start(pmax_d[:, :], pmax[0:1, 0:1])
            head = off - plan.chunks[-1][0]
            nc.scalar.dma_start(degw_d[:, head:], degsb[:, head:])

    nc.compile()
    return nc


# --------------------------------------------------------------------------
# launch B: out^T = WA^T X^T + WB^T (sX)^T + b, fp16/fp8, transposed layout
# --------------------------------------------------------------------------

def _build_main_program():
    """Launch B: outT = WA^T X + (WB/m)^T (t*X) + b.  The per-node factor t
    arrives pre-broadcast from the host (trep, pure replication), so the
    whole kernel is a straight-line DMA/compute pipeline with no on-device
    transpose or DRAM roundtrip."""
    nc = bacc.Bacc("TRN2", target_bir_lowering=False, debug=False,
                   num_devices=NCORES)
    A = mybir.AluOpType
    X = mybir.AxisListType.X

    xt_d = nc.dram_tensor("xt", [F, NPAD], fp16, kind="ExternalInput")
    wk_d = nc.dram_tensor("wk", [128, KCH * F], fp16, kind="ExternalInput")
    trep_d = nc.dram_tensor("trep", [F, NPAD], fp16, kind="ExternalInput")
    aux_d = nc.dram_tensor("aux", [128, 8], f32, kind="ExternalInput")
    out_d = nc.dram_tensor("out", [F, NPAD], fp16, kind="ExternalOutput")

    GW = 448                   # matmul group width (PSUM bank = 512 f32 max)
    # graduated chunk sizes (in GW groups): big first so the pipeline fills,
    # small last so little work trails the final DMA bytes
    GRP = [5, 5, 5, 4, 4, 3, 2]
    assert sum(GRP) * GW == NPAD
    XCH = len(GRP)
    CST = [sum(GRP[:i]) * GW for i in range(XCH + 1)]  # chunk col starts

    with tile.TileContext(nc) as tc:
        with (
            tc.tile_pool(name="const", bufs=1) as constp,
            tc.tile_pool(name="xt", bufs=1) as xtp,
            tc.tile_pool(name="trepp", bufs=1) as trepp,
            tc.tile_pool(name="outp", bufs=1) as outp,
            tc.tile_pool(name="small", bufs=1) as smallp,
            tc.tile_pool(name="sx", bufs=6) as sxp,
            tc.tile_pool(name="psf", bufs=8, space="PSUM") as psf,
        ):
            with tc.high_priority():
                aux_sb = smallp.tile([128, 8], f32)
                nc.sync.dma_start(aux_sb[:, :], aux_d[:, :])
                # ACT table pre-warm with a dependency-free input
                warm_in = smallp.tile([1, 1], f32)
                nc.vector.memset(warm_in[:, :], 0.0)
                nc.scalar.activation(warm_in[:, :], warm_in[:, :],
                                     mybir.ActivationFunctionType.Identity,
                                     bias=0.0, scale=1.0)
                ident16 = smallp.tile([128, 128], fp16)
                masks.make_identity(nc, ident16[:, :])
                # PE HAM warm-up: ~3us of dependency-free matmuls, abutting
                # the real stream, so it runs at 2.4 GHz
                ps_w = psf.tile([128, GW], f32, tag="psf")
                for _ in range(28):
                    nc.tensor.matmul(ps_w[:, 0:128], ident16[:, :], ident16[:, :],
                                     start=True, stop=True)
            wk_sb = constp.tile([128, KCH * F], fp16)
            nc.scalar.dma_start(wk_sb[:, :], wk_d[:, :])

            # ---- node features + t broadcast, interleaved on the sync ring -
            xt_sb, trep_sb = [], []
            for c in range(XCH):
                c0, c1 = CST[c], CST[c + 1]
                xt_c = xtp.tile([128, c1 - c0], fp16, name=f"xt{c}")
                nc.sync.dma_start(xt_c[:, :], xt_d[:, c0:c1])
                xt_sb.append(xt_c)
                trep_c = trepp.tile([128, c1 - c0], fp16, name=f"trep{c}")
                nc.sync.dma_start(trep_c[:, :], trep_d[:, c0:c1])
                trep_sb.append(trep_c)

            with tc.high_priority():
                minv = smallp.tile([128, 1], f32)
                nc.vector.reciprocal(minv[:, :], aux_sb[:, 0:1])
                btot_col = smallp.tile([128, 1], f32)
                nc.vector.tensor_reduce(btot_col[:, :], aux_sb[:, 1:8],
                                        axis=X, op=A.add)

            # ---- weights: WA | WB/m in fp16 --------------------------------
            # WA = W0 - W2 - 2W3 - 3W4 - 4W5,  WB = W1 + 2W2 + 3W3 + 4W4 + 5W5
            def wkk(k):
                return wk_sb[:, k * F : (k + 1) * F]
            wa16 = constp.tile([128, F], fp16)
            wb16 = constp.tile([128, F], fp16)
            nc.vector.scalar_tensor_tensor(wa16[:, :], wkk(2), -1.0, wkk(0), op0=A.mult, op1=A.add)
            nc.vector.scalar_tensor_tensor(wa16[:, :], wkk(3), -2.0, wa16[:, :], op0=A.mult, op1=A.add)
            nc.vector.scalar_tensor_tensor(wa16[:, :], wkk(4), -3.0, wa16[:, :], op0=A.mult, op1=A.add)
            nc.vector.scalar_tensor_tensor(wa16[:, :], wkk(5), -4.0, wa16[:, :], op0=A.mult, op1=A.add)
            nc.vector.scalar_tensor_tensor(wb16[:, :], wkk(2), 2.0, wkk(1), op0=A.mult, op1=A.add)
            nc.vector.scalar_tensor_tensor(wb16[:, :], wkk(3), 3.0, wb16[:, :], op0=A.mult, op1=A.add)
            nc.vector.scalar_tensor_tensor(wb16[:, :], wkk(4), 4.0, wb16[:, :], op0=A.mult, op1=A.add)
            nc.vector.scalar_tensor_tensor(wb16[:, :], wkk(5), 5.0, wb16[:, :], op0=A.mult, op1=A.add)
            nc.vector.tensor_scalar_mul(wb16[:, :], wb16[:, :], minv[:, 0:1])

            # ---- main loop -------------------------------------------------
            gidx = 0
            for c in range(XCH):
                c0, c1 = CST[c], CST[c + 1]
                out_c = outp.tile([128, c1 - c0], fp16, name=f"out{c}")
                sxs, psFs = [], []
                for g in range(GRP[c]):
                    n0 = g * GW
                    sx = sxp.tile([128, GW], fp16, tag="sx")
                    nc.vector.tensor_tensor(sx[:, :], xt_sb[c][:, n0 : n0 + GW],
                                            trep_sb[c][:, n0 : n0 + GW], op=A.mult)
                    sxs.append(sx)
                for g in range(GRP[c]):
                    n0 = g * GW
                    psF = psf.tile([128, GW], f32, tag="psf")
                    nc.tensor.matmul(psF[:, :], wa16[:, :],
                                     xt_sb[c][:, n0 : n0 + GW], start=True, stop=False)
                    psFs.append(psF)
                for g in range(GRP[c]):
                    nc.tensor.matmul(psFs[g][:, :], wb16[:, :], sxs[g][:, :],
                                     start=False, stop=True)
                for g in range(GRP[c]):
                    n0 = g * GW
                    # split evacuation between ACT and DVE
                    if gidx % 3 == 2:
                        nc.vector.tensor_scalar_add(out_c[:, n0 : n0 + GW],
                                                    psFs[g][:, :], btot_col[:, 0:1])
                    else:
                        nc.scalar.activation(out_c[:, n0 : n0 + GW], psFs[g][:, :],
                                             mybir.ActivationFunctionType.Identity,
                                             bias=btot_col[:, 0:1], scale=1.0)
                    gidx += 1
                nc.scalar.dma_start(out_d[:, c0:c1], out_c[:, :])

    nc.compile()
    return nc


# --------------------------------------------------------------------------
# host driver
# --------------------------------------------------------------------------

def kernel(nodes, edges, senders, receivers, Wk, bk, bias):
    nodes = np.ascontiguousarray(np.asarray(nodes, np.float32))
    edges = np.ascontiguousarray(np.asarray(edges, np.float32))
    senders = np.asarray(senders)
    receivers = np.asarray(receivers)
    Wk = np.ascontiguousarray(np.asarray(Wk, np.float32))
    bk = np.asarray(bk, np.float32)
    bias = np.asarray(bias, np.float32)
    assert nodes.shape == (N_NODES, F) and Wk.shape == (KCH, F, F)

    cores = list(range(NCORES))

    # ---- common packing layout across cores AND both buffers ---------------
    # (sender and receiver edges of a node share one slot assignment so the
    # device can accumulate in_w with negated masks: permutation + zero fill)
    lnS_c, lnR_c, wS_c, wR_c, cls_c = [], [], [], [], []
    for c in cores:
        mS = (senders // NPC) == c
        mR = (receivers // NPC) == c
        lnS = (senders[mS] - c * NPC).astype(np.int64)
        lnR = (receivers[mR] - c * NPC).astype(np.int64)
        lnS_c.append(lnS); lnR_c.append(lnR)
        wS_c.append(edges[mS]); wR_c.append(edges[mR])
        cnt = np.maximum(np.bincount(lnS, minlength=NPC),
                         np.bincount(lnR, minlength=NPC))
        cls_c.append(np.maximum(((np.maximum(cnt, 1) + 7) // 8) * 8, 8))
    plan = _CommonPlan(cls_c)

    ncls = len(plan.classes)
    MASKW = 64 * ncls
    maskblk = np.zeros((128, MASKW), np_fp8)
    for ci, cl in enumerate(plan.classes):
        k = 128 // cl
        for i in range(k):
            maskblk[i * cl : (i + 1) * cl, ci * 64 + i] = 1.0
            maskblk[i * cl : (i + 1) * cl, ci * 64 + 32 + i] = -1.0
    in_a = []
    evmaps = []
    for c in cores:
        data = np.zeros((128, MASKW + 2 * plan.ncols), np_fp8)
        data[:, :MASKW] = maskblk
        pS, cS = plan.fill(data, MASKW, lnS_c[c], wS_c[c], NPC, cls_c[c])
        pR, cR = plan.fill(data, MASKW + plan.ncols, lnR_c[c], wR_c[c], NPC, cls_c[c])
        assert np.array_equal(pS, pR) and np.array_equal(cS, cR)
        evmaps.append((pS, cS))
        in_a.append({"ed": np.ascontiguousarray(data)})

    key = ("edge", tuple(plan.classes), plan.ncols,
           tuple((W, tuple(map(tuple, g))) for W, g in plan.chunks))
    if key not in _prog_cache:
        _prog_cache[key] = _build_edge_program(plan)
    ncA = _prog_cache[key]

    res_a = run_bass_kernel_spmd(ncA, in_a, cores, trace=TRACE)

    # combine the 8 device partial maxima (selection, no arithmetic)
    m = max(float(res_a.results[c]["pmax"][0, 0]) for c in cores)

    # ---- host permutes packed sums into node order (selection only) -------
    if ("main",) not in _prog_cache:
        _prog_cache[("main",)] = _build_main_program()
    ncB = _prog_cache[("main",)]

    bkvec = np.concatenate([bk, bias.reshape(1, F)], axis=0)  # [7, F]
    wk16 = np.ascontiguousarray(
        Wk.transpose(1, 0, 2).reshape(128, KCH * F).astype(np.float16))
    in_b = []
    for c in cores:
        pS, cS = evmaps[c]
        dw = res_a.results[c]["degw"]                 # [128, EVC] fp16 (= t)
        trow = np.zeros(NPAD, np.float16)
        trow[:NPC] = dw[pS, cS]                       # node order (selection)
        trep = np.ascontiguousarray(np.broadcast_to(trow[None, :], (F, NPAD)))
        aux = np.zeros((128, 8), np.float32)
        aux[:, 0] = m
        aux[:, 1:8] = bkvec.T                          # [128(fo), 7]
        xt = np.zeros((F, NPAD), np.float16)
        xt[:, :NPC] = nodes[c * NPC : (c + 1) * NPC].T
        in_b.append({"xt": xt, "wk": wk16, "trep": trep, "aux": aux})
    res_b = run_bass_kernel_spmd(ncB, in_b, cores, trace=TRACE)

    ta = res_a.exec_time_ns
    tb = res_b.exec_time_ns
    LAST["exec_a_ns"] = ta
    LAST["exec_b_ns"] = tb
    LAST["exec_time_ns"] = (ta + tb) if (ta is not None and tb is not None) else None

    out = np.empty((N_NODES, F), np.float32)
    for c in cores:
        o = res_b.results[c]["out"]
        out[c * NPC : (c + 1) * NPC] = o.astype(np.float32).T[:NPC]
    return out
